# revision 8
# baseline (speedup 1.0000x reference)
"""Causal self-attention (B=2, T=4096, D=512, H=8) on 8 TRN2 NeuronCores.

Sharding: head/tensor parallel x data parallel. Core c (0..7) handles
batch b = c // 4 and head pair g = c % 4 (heads 2g, 2g+1). Each core
computes, for its batch and its two heads: the QKV projections, causal
flash attention over the full sequence, and a partial output projection
against its 128 columns of w_out. The host sums the four partial
[T, D] outputs per batch (the reduce step of the column-parallel
out-projection) and stacks the two batches.

On-chip layout ("transposed flash"): scores are computed as
S^T[k, q] = K^T_tile.T @ Q^T so softmax normalization reduces over the
PSUM partition axis via an appended ones-column on the V stationary
([V | 1]), which yields numerator rows 0..63 and the denominator in
row 64 of the same accumulator. exp() runs on the scalar engine with
the 1/sqrt(HD) scale folded in; causal masking is memset + affine_select
on the diagonal-straddling tiles only. All matmuls use float32r
(full-rate fp32 on the PE array, ~1.5e-4 rel err).
"""

import sys
import types
from contextlib import ExitStack

import numpy as np

B, T, D = 2, 4096, 512
H, HD = 8, 64
QB = 512  # query block (columns of S^T tiles)
KT = 128  # key tile (partition rows of S^T tiles)
NQB = T // QB  # 8
NKT = T // KT  # 32
EC = D // 128  # 4 contraction chunks of 128 over the model dim


def _install_ntff_shim():
    """Make ``antenv.axon_hooks`` importable so run_bass_kernel_spmd's
    trace path never crashes (and actually profiles when the axon .so
    supports it). Degrades to trace-skipped if anything is missing."""
    if "antenv.axon_hooks" in sys.modules:
        return
    mod = types.ModuleType("antenv.axon_hooks")
    mod._hook = None
    mod.set_axon_ntff_profile_hook = lambda h: setattr(mod, "_hook", h)
    mod.get_axon_ntff_profile_hook = lambda: mod._hook
    sys.modules["antenv.axon_hooks"] = mod
    try:
        import antenv

        antenv.axon_hooks = mod
    except ImportError:
        pass
    try:
        from trn_agent_boot.trn_boot import _ntff_profile_via_ctypes

        mod._hook = _ntff_profile_via_ctypes("/opt/axon/libaxon_pjrt.so")
    except Exception:
        pass


_NC_CACHE = {}


def _build():
    import concourse.bass as bass
    import concourse.mybir as mybir
    import concourse.tile as tile
    from concourse import bacc

    F32 = mybir.dt.float32
    F32R = mybir.dt.float32r
    EXP = mybir.ActivationFunctionType.Exp
    GE = mybir.AluOpType.is_ge

    nc = bacc.Bacc(None, target_bir_lowering=False)
    xT_in = nc.declare_dram_parameter("xT", [D, T], F32, isOutput=False)
    wqT_in = nc.declare_dram_parameter("wqT", [D, 128], F32, isOutput=False)
    wkT_in = nc.declare_dram_parameter("wkT", [D, 128], F32, isOutput=False)
    wvT_in = nc.declare_dram_parameter("wvT", [D, 128], F32, isOutput=False)
    woT_in = nc.declare_dram_parameter("woT", [128, D], F32, isOutput=False)
    y_out = nc.declare_dram_parameter("y", [T, D], F32, isOutput=True)

    with tile.TileContext(nc) as tc, ExitStack() as ctx:
        const = ctx.enter_context(tc.tile_pool(name="const", bufs=1))
        big = ctx.enter_context(tc.tile_pool(name="big", bufs=1))

        # [128, 64] identity in both partition halves so transposes of
        # operands based at partition 0 or 64 can use a matching slice
        ident = const.tile([128, 64], F32)
        nc.gpsimd.memset(ident[:], 0.0)
        for half in range(2):
            nc.gpsimd.affine_select(
                out=ident[bass.ts(half, 64), :],
                in_=ident[bass.ts(half, 64), :],
                compare_op=mybir.AluOpType.not_equal,
                fill=1.0,
                base=0,
                pattern=[[-1, 64]],
                channel_multiplier=1,
            )

        # Causal masks for the 4 diagonal-straddling k-tile offsets.
        # mask[k, d, q] = 1.0 iff (q - k - d*KT) >= 0, else 0.
        cmask = const.tile([128, QB // KT, QB], F32)
        nc.gpsimd.memset(cmask[:], 1.0)
        for di in range(QB // KT):
            nc.gpsimd.affine_select(
                out=cmask[:, di, :],
                in_=cmask[:, di, :],
                compare_op=GE,
                fill=0.0,
                base=-di * KT,
                pattern=[[1, QB]],
                channel_multiplier=-1,
            )

        # ---- prologue: weights, x^T, projections, V tiles ----
        qT_r = big.tile([128, T], F32R)  # partitions: head A d 0-63, head B 64-127
        kT_r = big.tile([128, T], F32R)
        xT_r = big.tile([128, EC, T], F32R)
        v_t = big.tile([128, 2, NKT, 65], F32R)
        wo_r = const.tile([128, D], F32R)

        with (
            tc.tile_pool(name="stage", bufs=1) as stage,
            tc.tile_pool(name="xstage", bufs=2) as xstage,
            tc.tile_pool(name="proj_ps", bufs=2, space="PSUM") as proj_ps,
        ):
            w_f = stage.tile([128, 3, EC, 128], F32)  # staging for wq/wk/wv T
            nc.sync.dma_start(w_f[:, 0], wqT_in.rearrange("(c p) d -> p c d", p=128))
            nc.sync.dma_start(w_f[:, 1], wkT_in.rearrange("(c p) d -> p c d", p=128))
            nc.sync.dma_start(w_f[:, 2], wvT_in.rearrange("(c p) d -> p c d", p=128))
            w_r = stage.tile([128, 3, EC, 128], F32R)
            nc.vector.tensor_copy(w_r[:], w_f[:])

            wo_f = stage.tile([128, D], F32)
            nc.sync.dma_start(wo_f[:], woT_in[:])
            nc.vector.tensor_copy(wo_r[:], wo_f[:])

            # x^T: DMA + round to f32r, [128, EC, T]
            xT_view = xT_in.rearrange("(c p) t -> p c t", p=128)
            for s in range(NQB):
                xs = xstage.tile([128, EC, QB], F32)
                nc.sync.dma_start(xs[:], xT_view[:, :, bass.ts(s, QB)])
                nc.vector.tensor_copy(xT_r[:, :, bass.ts(s, QB)], xs[:])

            # projections: Q^T, K^T (f32r) and V^T (f32)
            vT_f = stage.tile([128, T], F32)
            for wi, dst in enumerate((qT_r, kT_r, vT_f)):
                for s in range(NQB):
                    pt = proj_ps.tile([128, QB], F32, tag="proj")
                    for c in range(EC):
                        nc.tensor.matmul(
                            pt[:],
                            w_r[:, wi, c],
                            xT_r[:, c, bass.ts(s, QB)],
                            start=(c == 0),
                            stop=(c == EC - 1),
                        )
                    nc.vector.tensor_copy(dst[:, bass.ts(s, QB)], pt[:])

            # ---- V tiles: [128 k, 65] per (head, k-tile), ones col appended
            # (f32r memset is not a valid ISA op; round a f32 ones tile)
            ones_f = stage.tile([128, NKT], F32)
            nc.vector.memset(ones_f[:], 1.0)
            for h in range(2):
                nc.vector.tensor_copy(
                    v_t[:, h, :, 64:65], ones_f[:].unsqueeze(2)
                )
            for h in range(2):
                for t in range(NKT):
                    pv = proj_ps.tile([128, 64], F32, tag="vtp")
                    nc.tensor.transpose(
                        pv[:],
                        vT_f[bass.ts(h, 64), bass.ts(t, KT)],
                        ident[bass.ts(h, 64), :],
                    )
                    nc.vector.tensor_copy(v_t[:, h, t, 0:64], pv[:])

        # ---- attention + out-projection ----
        s_ps = ctx.enter_context(tc.tile_pool(name="s_ps", bufs=2, space="PSUM"))
        acc_ps = ctx.enter_context(tc.tile_pool(name="acc_ps", bufs=2, space="PSUM"))
        y_ps = ctx.enter_context(tc.tile_pool(name="y_ps", bufs=2, space="PSUM"))
        p_sb = ctx.enter_context(tc.tile_pool(name="p_sb", bufs=6))
        a_sb = ctx.enter_context(tc.tile_pool(name="a_sb", bufs=2))
        d_sb = ctx.enter_context(tc.tile_pool(name="d_sb", bufs=2))
        y_sb = ctx.enter_context(tc.tile_pool(name="y_sb", bufs=3))

        scale = 1.0 / float(np.sqrt(HD))
        for J in range(NQB):
            aT_st = a_sb.tile([128, QB], F32R)
            for h in range(2):
                acc = acc_ps.tile([65, QB], F32)
                ktiles = (J + 1) * (QB // KT)
                for t in range(ktiles):
                    st = s_ps.tile([128, QB], F32)
                    nc.tensor.matmul(
                        st[:],
                        kT_r[bass.ts(h, 64), bass.ts(t, KT)],
                        qT_r[bass.ts(h, 64), bass.ts(J, QB)],
                        start=True,
                        stop=True,
                    )
                    pt = p_sb.tile([128, QB], F32R)
                    nc.scalar.activation(pt[:], st[:], EXP, scale=scale)
                    diag = t * KT - J * QB  # k-tile offset within the q block
                    if diag >= 0:
                        # columns beyond diag+KT are fully unmasked
                        w = diag + KT
                        nc.vector.tensor_mul(
                            pt[:, 0:w],
                            pt[:, 0:w],
                            cmask[:, diag // KT, 0:w],
                        )
                    nc.tensor.matmul(
                        acc[:],
                        v_t[:, h, t],
                        pt[:],
                        start=(t == 0),
                        stop=(t == ktiles - 1),
                    )
                # numerator rows 0..63, denominator row 64 -> divide
                drow = d_sb.tile([1, QB], F32, tag="drow")
                nc.vector.tensor_copy(drow[:], acc[64:65, :])
                rrow = d_sb.tile([1, QB], F32, tag="rrow")
                nc.vector.reciprocal(rrow[:], drow[:])
                bc = d_sb.tile([64, QB], F32, tag="bc")
                nc.gpsimd.partition_broadcast(bc[:], rrow[:])
                nc.vector.tensor_mul(
                    aT_st[bass.ts(h, 64), :], acc[0:64, :], bc[:]
                )
            for sub in range(QB // KT):
                yp = y_ps.tile([128, D], F32)
                nc.tensor.matmul(
                    yp[:],
                    aT_st[:, bass.ts(sub, 128)],
                    wo_r[:],
                    start=True,
                    stop=True,
                )
                ysb = y_sb.tile([128, D], F32)
                nc.vector.tensor_copy(ysb[:], yp[:])
                nc.sync.dma_start(
                    y_out[bass.ds(J * QB + sub * 128, 128), :], ysb[:]
                )

    nc.compile()
    return nc


def get_nc():
    if "nc" not in _NC_CACHE:
        _NC_CACHE["nc"] = _build()
    return _NC_CACHE["nc"]


def make_in_maps(x, w_qkv, w_out):
    x = np.ascontiguousarray(np.asarray(x), dtype=np.float32)
    w_qkv = np.ascontiguousarray(np.asarray(w_qkv), dtype=np.float32)
    w_out = np.ascontiguousarray(np.asarray(w_out), dtype=np.float32)
    in_maps = []
    for c in range(8):
        b, g = divmod(c, 4)
        rows = slice(g * 128, (g + 1) * 128)
        in_maps.append(
            {
                "xT": np.ascontiguousarray(x[b].T),
                "wqT": np.ascontiguousarray(w_qkv[rows, :].T),
                "wkT": np.ascontiguousarray(w_qkv[512 + g * 128 : 512 + (g + 1) * 128, :].T),
                "wvT": np.ascontiguousarray(w_qkv[1024 + g * 128 : 1024 + (g + 1) * 128, :].T),
                "woT": np.ascontiguousarray(w_out[:, rows].T),
            }
        )
    return in_maps


def combine_results(results):
    y = np.zeros((B, T, D), dtype=np.float32)
    for c, r in enumerate(results):
        y[c // 4] += r["y"]
    return y


def kernel(x, w_qkv, w_out, trace=False):
    _install_ntff_shim()
    from concourse.bass_utils import run_bass_kernel_spmd

    nc = get_nc()
    in_maps = make_in_maps(x, w_qkv, w_out)
    r = run_bass_kernel_spmd(nc, in_maps, core_ids=list(range(8)), trace=trace)
    y = combine_results(r.results)
    if trace:
        return y, r
    return y


# revision 9
# speedup vs baseline: 1.1417x; 1.1417x over previous
"""Causal self-attention (B=2, T=4096, D=512, H=8) on 8 TRN2 NeuronCores.

Sharding: head/tensor parallel x data parallel. Core c (0..7) handles
batch b = c // 4 and head pair g = c % 4 (heads 2g, 2g+1). Each core
computes, for its batch and its two heads: the QKV projections, causal
flash attention over the full sequence, and a partial output projection
against its 128 columns of w_out. The host sums the four partial
[T, D] outputs per batch (the reduce step of the column-parallel
out-projection) and stacks the two batches.

On-chip layout ("transposed flash"): scores are computed as
S^T[k, q] = K^T_tile.T @ Q^T so softmax normalization reduces over the
PSUM partition axis via an appended ones-column on the V stationary
([V | 1]), which yields numerator rows 0..63 and the denominator in
row 64 of the same accumulator. exp() runs on the scalar engine with
the 1/sqrt(HD) scale folded in; causal masking is memset + affine_select
on the diagonal-straddling tiles only. All matmuls use float32r
float16 operands (full PE rate, FWL weight loads, ~5e-4 rel err; accumulation is fp32 in PSUM).
"""

import sys
import types
from contextlib import ExitStack

import numpy as np

B, T, D = 2, 4096, 512
H, HD = 8, 64
QB = 512  # query block (columns of S^T tiles)
KT = 128  # key tile (partition rows of S^T tiles)
NQB = T // QB  # 8
NKT = T // KT  # 32
EC = D // 128  # 4 contraction chunks of 128 over the model dim


def _install_ntff_shim():
    """Make ``antenv.axon_hooks`` importable so run_bass_kernel_spmd's
    trace path never crashes (and actually profiles when the axon .so
    supports it). Degrades to trace-skipped if anything is missing."""
    if "antenv.axon_hooks" in sys.modules:
        return
    mod = types.ModuleType("antenv.axon_hooks")
    mod._hook = None
    mod.set_axon_ntff_profile_hook = lambda h: setattr(mod, "_hook", h)
    mod.get_axon_ntff_profile_hook = lambda: mod._hook
    sys.modules["antenv.axon_hooks"] = mod
    try:
        import antenv

        antenv.axon_hooks = mod
    except ImportError:
        pass
    try:
        from trn_agent_boot.trn_boot import _ntff_profile_via_ctypes

        mod._hook = _ntff_profile_via_ctypes("/opt/axon/libaxon_pjrt.so")
    except Exception:
        pass


_NC_CACHE = {}


def _build():
    import concourse.bass as bass
    import concourse.mybir as mybir
    import concourse.tile as tile
    from concourse import bacc

    F32 = mybir.dt.float32
    F32R = mybir.dt.float16
    EXP = mybir.ActivationFunctionType.Exp
    GE = mybir.AluOpType.is_ge

    nc = bacc.Bacc(None, target_bir_lowering=False)
    xT_in = nc.declare_dram_parameter("xT", [D, T], F32, isOutput=False)
    wqT_in = nc.declare_dram_parameter("wqT", [D, 128], F32, isOutput=False)
    wkT_in = nc.declare_dram_parameter("wkT", [D, 128], F32, isOutput=False)
    wvT_in = nc.declare_dram_parameter("wvT", [D, 128], F32, isOutput=False)
    woT_in = nc.declare_dram_parameter("woT", [128, D], F32, isOutput=False)
    y_out = nc.declare_dram_parameter("y", [T, D], F32, isOutput=True)

    with tile.TileContext(nc) as tc, ExitStack() as ctx:
        const = ctx.enter_context(tc.tile_pool(name="const", bufs=1))
        big = ctx.enter_context(tc.tile_pool(name="big", bufs=1))

        # [128, 64] identity in both partition halves so transposes of
        # operands based at partition 0 or 64 can use a matching slice
        ident = const.tile([128, 64], F32)
        nc.gpsimd.memset(ident[:], 0.0)
        for half in range(2):
            nc.gpsimd.affine_select(
                out=ident[bass.ts(half, 64), :],
                in_=ident[bass.ts(half, 64), :],
                compare_op=mybir.AluOpType.not_equal,
                fill=1.0,
                base=0,
                pattern=[[-1, 64]],
                channel_multiplier=1,
            )

        # Causal masks for the 4 diagonal-straddling k-tile offsets.
        # mask[k, d, q] = 1.0 iff (q - k - d*KT) >= 0, else 0.
        cmask = const.tile([128, QB // KT, QB], F32R)
        nc.gpsimd.memset(cmask[:], 1.0)
        for di in range(QB // KT):
            nc.gpsimd.affine_select(
                out=cmask[:, di, :],
                in_=cmask[:, di, :],
                compare_op=GE,
                fill=0.0,
                base=-di * KT,
                pattern=[[1, QB]],
                channel_multiplier=-1,
            )

        # ---- prologue: weights, x^T, projections, V tiles ----
        qT_r = big.tile([128, T], F32R)  # partitions: head A d 0-63, head B 64-127
        kT_r = big.tile([128, T], F32R)
        xT_r = big.tile([128, EC, T], F32R)
        v_t = big.tile([128, 2, NKT, 65], F32R)
        wo_r = const.tile([128, D], F32R)

        with (
            tc.tile_pool(name="stage", bufs=1) as stage,
            tc.tile_pool(name="xstage", bufs=2) as xstage,
            tc.tile_pool(name="proj_ps", bufs=2, space="PSUM") as proj_ps,
        ):
            w_f = stage.tile([128, 3, EC, 128], F32)  # staging for wq/wk/wv T
            nc.sync.dma_start(w_f[:, 0], wqT_in.rearrange("(c p) d -> p c d", p=128))
            nc.sync.dma_start(w_f[:, 1], wkT_in.rearrange("(c p) d -> p c d", p=128))
            nc.sync.dma_start(w_f[:, 2], wvT_in.rearrange("(c p) d -> p c d", p=128))
            w_r = stage.tile([128, 3, EC, 128], F32R)
            nc.vector.tensor_copy(w_r[:], w_f[:])

            wo_f = stage.tile([128, D], F32)
            nc.sync.dma_start(wo_f[:], woT_in[:])
            nc.vector.tensor_copy(wo_r[:], wo_f[:])

            # x^T: DMA + round to f32r, [128, EC, T]
            xT_view = xT_in.rearrange("(c p) t -> p c t", p=128)
            for s in range(NQB):
                xs = xstage.tile([128, EC, QB], F32)
                nc.sync.dma_start(xs[:], xT_view[:, :, bass.ts(s, QB)])
                nc.vector.tensor_copy(xT_r[:, :, bass.ts(s, QB)], xs[:])

            # projections: Q^T, K^T (f32r) and V^T (f32)
            vT_f = stage.tile([128, T], F32)
            for wi, dst in enumerate((qT_r, kT_r, vT_f)):
                for s in range(NQB):
                    pt = proj_ps.tile([128, QB], F32, tag="proj")
                    for c in range(EC):
                        nc.tensor.matmul(
                            pt[:],
                            w_r[:, wi, c],
                            xT_r[:, c, bass.ts(s, QB)],
                            start=(c == 0),
                            stop=(c == EC - 1),
                        )
                    nc.vector.tensor_copy(dst[:, bass.ts(s, QB)], pt[:])

            # ---- V tiles: [128 k, 65] per (head, k-tile), ones col appended
            # (f32r memset is not a valid ISA op; round a f32 ones tile)
            ones_f = stage.tile([128, NKT], F32)
            nc.vector.memset(ones_f[:], 1.0)
            for h in range(2):
                nc.vector.tensor_copy(
                    v_t[:, h, :, 64:65], ones_f[:].unsqueeze(2)
                )
            for h in range(2):
                for t in range(NKT):
                    pv = proj_ps.tile([128, 64], F32, tag="vtp")
                    nc.tensor.transpose(
                        pv[:],
                        vT_f[bass.ts(h, 64), bass.ts(t, KT)],
                        ident[bass.ts(h, 64), :],
                    )
                    nc.vector.tensor_copy(v_t[:, h, t, 0:64], pv[:])

        # ---- attention + out-projection ----
        s_ps = ctx.enter_context(tc.tile_pool(name="s_ps", bufs=2, space="PSUM"))
        acc_ps = ctx.enter_context(tc.tile_pool(name="acc_ps", bufs=2, space="PSUM"))
        y_ps = ctx.enter_context(tc.tile_pool(name="y_ps", bufs=2, space="PSUM"))
        p_sb = ctx.enter_context(tc.tile_pool(name="p_sb", bufs=6))
        a_sb = ctx.enter_context(tc.tile_pool(name="a_sb", bufs=2))
        d_sb = ctx.enter_context(tc.tile_pool(name="d_sb", bufs=2))
        y_sb = ctx.enter_context(tc.tile_pool(name="y_sb", bufs=3))

        scale = 1.0 / float(np.sqrt(HD))
        for J in range(NQB):
            aT_st = a_sb.tile([128, QB], F32R)
            for h in range(2):
                acc = acc_ps.tile([65, QB], F32)
                ktiles = (J + 1) * (QB // KT)
                for t in range(ktiles):
                    st = s_ps.tile([128, QB], F32)
                    nc.tensor.matmul(
                        st[:],
                        kT_r[bass.ts(h, 64), bass.ts(t, KT)],
                        qT_r[bass.ts(h, 64), bass.ts(J, QB)],
                        start=True,
                        stop=True,
                    )
                    pt = p_sb.tile([128, QB], F32R)
                    nc.scalar.activation(pt[:], st[:], EXP, scale=scale)
                    diag = t * KT - J * QB  # k-tile offset within the q block
                    if diag >= 0:
                        # columns beyond diag+KT are fully unmasked
                        w = diag + KT
                        nc.vector.tensor_mul(
                            pt[:, 0:w],
                            pt[:, 0:w],
                            cmask[:, diag // KT, 0:w],
                        )
                    nc.tensor.matmul(
                        acc[:],
                        v_t[:, h, t],
                        pt[:],
                        start=(t == 0),
                        stop=(t == ktiles - 1),
                    )
                # numerator rows 0..63, denominator row 64 -> divide
                drow = d_sb.tile([1, QB], F32, tag="drow")
                nc.vector.tensor_copy(drow[:], acc[64:65, :])
                bc = d_sb.tile([64, QB], F32, tag="bc")
                nc.gpsimd.partition_broadcast(bc[:], drow[:])
                rbc = d_sb.tile([64, QB], F32, tag="rbc")
                nc.vector.reciprocal(rbc[:], bc[:])
                nc.vector.tensor_mul(
                    aT_st[bass.ts(h, 64), :], acc[0:64, :], rbc[:]
                )
            for sub in range(QB // KT):
                yp = y_ps.tile([128, D], F32)
                nc.tensor.matmul(
                    yp[:],
                    aT_st[:, bass.ts(sub, 128)],
                    wo_r[:],
                    start=True,
                    stop=True,
                )
                ysb = y_sb.tile([128, D], F32)
                nc.vector.tensor_copy(ysb[:], yp[:])
                nc.sync.dma_start(
                    y_out[bass.ds(J * QB + sub * 128, 128), :], ysb[:]
                )

    nc.compile()
    return nc


def get_nc():
    if "nc" not in _NC_CACHE:
        _NC_CACHE["nc"] = _build()
    return _NC_CACHE["nc"]


def make_in_maps(x, w_qkv, w_out):
    x = np.ascontiguousarray(np.asarray(x), dtype=np.float32)
    w_qkv = np.ascontiguousarray(np.asarray(w_qkv), dtype=np.float32)
    w_out = np.ascontiguousarray(np.asarray(w_out), dtype=np.float32)
    in_maps = []
    for c in range(8):
        b, g = divmod(c, 4)
        rows = slice(g * 128, (g + 1) * 128)
        in_maps.append(
            {
                "xT": np.ascontiguousarray(x[b].T),
                "wqT": np.ascontiguousarray(w_qkv[rows, :].T),
                "wkT": np.ascontiguousarray(w_qkv[512 + g * 128 : 512 + (g + 1) * 128, :].T),
                "wvT": np.ascontiguousarray(w_qkv[1024 + g * 128 : 1024 + (g + 1) * 128, :].T),
                "woT": np.ascontiguousarray(w_out[:, rows].T),
            }
        )
    return in_maps


def combine_results(results):
    y = np.zeros((B, T, D), dtype=np.float32)
    for c, r in enumerate(results):
        y[c // 4] += r["y"]
    return y


def kernel(x, w_qkv, w_out, trace=False):
    _install_ntff_shim()
    from concourse.bass_utils import run_bass_kernel_spmd

    nc = get_nc()
    in_maps = make_in_maps(x, w_qkv, w_out)
    r = run_bass_kernel_spmd(nc, in_maps, core_ids=list(range(8)), trace=trace)
    y = combine_results(r.results)
    if trace:
        return y, r
    return y


# revision 11
# speedup vs baseline: 1.1758x; 1.0299x over previous
"""Causal self-attention (B=2, T=4096, D=512, H=8) on 8 TRN2 NeuronCores.

Sharding: head/tensor parallel x data parallel. Core c (0..7) handles
batch b = c // 4 and head pair g = c % 4 (heads 2g, 2g+1). Each core
computes, for its batch and its two heads: the QKV projections, causal
flash attention over the full sequence, and a partial output projection
against its 128 columns of w_out. The host sums the four partial
[T, D] outputs per batch (the reduce step of the column-parallel
out-projection) and stacks the two batches.

On-chip layout ("transposed flash"): scores are computed as
S^T[k, q] = K^T_tile.T @ Q^T so softmax normalization reduces over the
PSUM partition axis via an appended ones-column on the V stationary
([V | 1]), which yields numerator rows 0..63 and the denominator in
row 64 of the same accumulator. exp() runs on the scalar engine with
the 1/sqrt(HD) scale folded in; causal masking is memset + affine_select
on the diagonal-straddling tiles only. All matmuls use float32r
float16 operands (full PE rate, FWL weight loads, ~5e-4 rel err; accumulation is fp32 in PSUM).
"""

import sys
import types
from contextlib import ExitStack

import numpy as np

B, T, D = 2, 4096, 512
H, HD = 8, 64
QB = 512  # query block (columns of S^T tiles)
KT = 128  # key tile (partition rows of S^T tiles)
NQB = T // QB  # 8
NKT = T // KT  # 32
EC = D // 128  # 4 contraction chunks of 128 over the model dim


def _install_ntff_shim():
    """Make ``antenv.axon_hooks`` importable so run_bass_kernel_spmd's
    trace path never crashes (and actually profiles when the axon .so
    supports it). Degrades to trace-skipped if anything is missing."""
    if "antenv.axon_hooks" in sys.modules:
        return
    mod = types.ModuleType("antenv.axon_hooks")
    mod._hook = None
    mod.set_axon_ntff_profile_hook = lambda h: setattr(mod, "_hook", h)
    mod.get_axon_ntff_profile_hook = lambda: mod._hook
    sys.modules["antenv.axon_hooks"] = mod
    try:
        import antenv

        antenv.axon_hooks = mod
    except ImportError:
        pass
    try:
        from trn_agent_boot.trn_boot import _ntff_profile_via_ctypes

        mod._hook = _ntff_profile_via_ctypes("/opt/axon/libaxon_pjrt.so")
    except Exception:
        pass


_NC_CACHE = {}


def _build():
    import concourse.bass as bass
    import concourse.mybir as mybir
    import concourse.tile as tile
    from concourse import bacc

    F32 = mybir.dt.float32
    F32R = mybir.dt.float16
    EXP = mybir.ActivationFunctionType.Exp
    GE = mybir.AluOpType.is_ge

    nc = bacc.Bacc(None, target_bir_lowering=False)
    xT_in = nc.declare_dram_parameter("xT", [D, T], F32, isOutput=False)
    wqT_in = nc.declare_dram_parameter("wqT", [D, 128], F32, isOutput=False)
    wkT_in = nc.declare_dram_parameter("wkT", [D, 128], F32, isOutput=False)
    wvT_in = nc.declare_dram_parameter("wvT", [D, 128], F32, isOutput=False)
    woT_in = nc.declare_dram_parameter("woT", [128, D], F32, isOutput=False)
    y_out = nc.declare_dram_parameter("y", [T, D], F32, isOutput=True)

    with tile.TileContext(nc) as tc, ExitStack() as ctx:
        const = ctx.enter_context(tc.tile_pool(name="const", bufs=1))
        big = ctx.enter_context(tc.tile_pool(name="big", bufs=1))

        # [128, 64] identity in both partition halves so transposes of
        # operands based at partition 0 or 64 can use a matching slice
        ident = const.tile([128, 64], F32)
        nc.gpsimd.memset(ident[:], 0.0)
        for half in range(2):
            nc.gpsimd.affine_select(
                out=ident[bass.ts(half, 64), :],
                in_=ident[bass.ts(half, 64), :],
                compare_op=mybir.AluOpType.not_equal,
                fill=1.0,
                base=0,
                pattern=[[-1, 64]],
                channel_multiplier=1,
            )

        # Causal masks for the 4 diagonal-straddling k-tile offsets.
        # mask[k, d, q] = 1.0 iff (q - k - d*KT) >= 0, else 0.
        cmask = const.tile([128, QB // KT, QB], F32R)
        nc.gpsimd.memset(cmask[:], 1.0)
        for di in range(QB // KT):
            nc.gpsimd.affine_select(
                out=cmask[:, di, :],
                in_=cmask[:, di, :],
                compare_op=GE,
                fill=0.0,
                base=-di * KT,
                pattern=[[1, QB]],
                channel_multiplier=-1,
            )

        # ---- prologue: weights, x^T, projections, V tiles ----
        qT_r = big.tile([128, T], F32R)  # partitions: head A d 0-63, head B 64-127
        kT_r = big.tile([128, T], F32R)
        xT_r = big.tile([128, EC, T], F32R)
        v_t = big.tile([128, 2, NKT, 65], F32R)
        wo_r = const.tile([128, D], F32R)

        with (
            tc.tile_pool(name="stage", bufs=1) as stage,
            tc.tile_pool(name="xstage", bufs=2) as xstage,
            tc.tile_pool(name="proj_ps", bufs=2, space="PSUM") as proj_ps,
        ):
            w_f = stage.tile([128, 3, EC, 128], F32)  # staging for wq/wk/wv T
            nc.sync.dma_start(w_f[:, 0], wqT_in.rearrange("(c p) d -> p c d", p=128))
            nc.sync.dma_start(w_f[:, 1], wkT_in.rearrange("(c p) d -> p c d", p=128))
            nc.sync.dma_start(w_f[:, 2], wvT_in.rearrange("(c p) d -> p c d", p=128))
            w_r = stage.tile([128, 3, EC, 128], F32R)
            nc.vector.tensor_copy(w_r[:], w_f[:])

            wo_f = stage.tile([128, D], F32)
            nc.sync.dma_start(wo_f[:], woT_in[:])
            nc.vector.tensor_copy(wo_r[:], wo_f[:])

            # x^T: per-e-chunk DMAs (16KB contiguous runs) + rounding casts
            for c in range(EC):
                xs = xstage.tile([128, T], F32, tag="xs")
                nc.sync.dma_start(xs[:], xT_in[bass.ts(c, 128), :])
                nc.vector.tensor_copy(xT_r[:, c, :], xs[:])

            # projections: Q^T, K^T (f32r) and V^T (f32)
            vT_f = stage.tile([128, T], F32)
            for wi, dst in enumerate((qT_r, kT_r, vT_f)):
                for s in range(NQB):
                    pt = proj_ps.tile([128, QB], F32, tag="proj")
                    for c in range(EC):
                        nc.tensor.matmul(
                            pt[:],
                            w_r[:, wi, c],
                            xT_r[:, c, bass.ts(s, QB)],
                            start=(c == 0),
                            stop=(c == EC - 1),
                        )
                    nc.vector.tensor_copy(dst[:, bass.ts(s, QB)], pt[:])

            # ---- V tiles: [128 k, 65] per (head, k-tile), ones col appended
            # (f32r memset is not a valid ISA op; round a f32 ones tile)
            ones_f = stage.tile([128, NKT], F32)
            nc.vector.memset(ones_f[:], 1.0)
            for h in range(2):
                nc.vector.tensor_copy(
                    v_t[:, h, :, 64:65], ones_f[:].unsqueeze(2)
                )
            for h in range(2):
                for t in range(NKT):
                    pv = proj_ps.tile([128, 64], F32, tag="vtp")
                    nc.tensor.transpose(
                        pv[:],
                        vT_f[bass.ts(h, 64), bass.ts(t, KT)],
                        ident[bass.ts(h, 64), :],
                    )
                    nc.vector.tensor_copy(v_t[:, h, t, 0:64], pv[:])

        # ---- attention + out-projection ----
        s_ps = ctx.enter_context(tc.tile_pool(name="s_ps", bufs=3, space="PSUM"))
        acc_ps = ctx.enter_context(tc.tile_pool(name="acc_ps", bufs=2, space="PSUM"))
        y_ps = ctx.enter_context(tc.tile_pool(name="y_ps", bufs=2, space="PSUM"))
        p_sb = ctx.enter_context(tc.tile_pool(name="p_sb", bufs=6))
        a_sb = ctx.enter_context(tc.tile_pool(name="a_sb", bufs=2))
        d_sb = ctx.enter_context(tc.tile_pool(name="d_sb", bufs=2))
        y_sb = ctx.enter_context(tc.tile_pool(name="y_sb", bufs=3))

        scale = 1.0 / float(np.sqrt(HD))
        DEPTH = 2  # how many k-tiles QK runs ahead of AV

        def emit_outproj(aT_prev, Jp, sub):
            yp = y_ps.tile([128, D], F32)
            nc.tensor.matmul(
                yp[:],
                aT_prev[:, bass.ts(sub, 128)],
                wo_r[:],
                start=True,
                stop=True,
            )
            ysb = y_sb.tile([128, D], F32)
            nc.vector.tensor_copy(ysb[:], yp[:])
            nc.sync.dma_start(
                y_out[bass.ds(Jp * QB + sub * 128, 128), :], ysb[:]
            )

        pending = []  # deferred out-proj thunks for the previous q-block
        for J in range(NQB):
            aT_st = a_sb.tile([128, QB], F32R)
            for h in range(2):
                acc = acc_ps.tile([65, QB], F32)
                ktiles = (J + 1) * (QB // KT)
                pts = [None] * ktiles
                for t in range(ktiles + DEPTH):
                    if t < ktiles:
                        st = s_ps.tile([128, QB], F32)
                        nc.tensor.matmul(
                            st[:],
                            kT_r[bass.ts(h, 64), bass.ts(t, KT)],
                            qT_r[bass.ts(h, 64), bass.ts(J, QB)],
                            start=True,
                            stop=True,
                        )
                        pt = p_sb.tile([128, QB], F32R)
                        nc.scalar.activation(pt[:], st[:], EXP, scale=scale)
                        diag = t * KT - J * QB
                        if diag >= 0:
                            w = diag + KT
                            nc.vector.tensor_mul(
                                pt[:, 0:w],
                                pt[:, 0:w],
                                cmask[:, diag // KT, 0:w],
                            )
                        pts[t] = pt
                        if pending and h == 0 and t % 2 == 1:
                            pending.pop(0)()
                    if t >= DEPTH:
                        nc.tensor.matmul(
                            acc[:],
                            v_t[:, h, t - DEPTH],
                            pts[t - DEPTH][:],
                            start=(t == DEPTH),
                            stop=(t == ktiles + DEPTH - 1),
                        )
                # numerator rows 0..63, denominator row 64 -> divide
                drow = d_sb.tile([1, QB], F32, tag="drow")
                nc.vector.tensor_copy(drow[:], acc[64:65, :])
                bc = d_sb.tile([64, QB], F32, tag="bc")
                nc.gpsimd.partition_broadcast(bc[:], drow[:])
                rbc = d_sb.tile([64, QB], F32, tag="rbc")
                nc.vector.reciprocal_approx_fast(out=rbc[:], in_=bc[:])
                nc.vector.tensor_mul(
                    aT_st[bass.ts(h, 64), :], acc[0:64, :], rbc[:]
                )
            for fl in pending:  # anything not yet emitted (early small blocks)
                fl()
            pending = [
                (lambda a=aT_st, Jp=J, sb=sub: emit_outproj(a, Jp, sb))
                for sub in range(QB // KT)
            ]
        for fl in pending:
            fl()

    nc.compile()
    return nc


def get_nc():
    if "nc" not in _NC_CACHE:
        _NC_CACHE["nc"] = _build()
    return _NC_CACHE["nc"]


def make_in_maps(x, w_qkv, w_out):
    x = np.ascontiguousarray(np.asarray(x), dtype=np.float32)
    w_qkv = np.ascontiguousarray(np.asarray(w_qkv), dtype=np.float32)
    w_out = np.ascontiguousarray(np.asarray(w_out), dtype=np.float32)
    in_maps = []
    for c in range(8):
        b, g = divmod(c, 4)
        rows = slice(g * 128, (g + 1) * 128)
        in_maps.append(
            {
                "xT": np.ascontiguousarray(x[b].T),
                "wqT": np.ascontiguousarray(w_qkv[rows, :].T),
                "wkT": np.ascontiguousarray(w_qkv[512 + g * 128 : 512 + (g + 1) * 128, :].T),
                "wvT": np.ascontiguousarray(w_qkv[1024 + g * 128 : 1024 + (g + 1) * 128, :].T),
                "woT": np.ascontiguousarray(w_out[:, rows].T),
            }
        )
    return in_maps


def combine_results(results):
    y = np.zeros((B, T, D), dtype=np.float32)
    for c, r in enumerate(results):
        y[c // 4] += r["y"]
    return y


def kernel(x, w_qkv, w_out, trace=False):
    _install_ntff_shim()
    from concourse.bass_utils import run_bass_kernel_spmd

    nc = get_nc()
    in_maps = make_in_maps(x, w_qkv, w_out)
    r = run_bass_kernel_spmd(nc, in_maps, core_ids=list(range(8)), trace=trace)
    y = combine_results(r.results)
    if trace:
        return y, r
    return y


# revision 22
# speedup vs baseline: 1.8425x; 1.5669x over previous
"""Causal self-attention (B=2, T=4096, D=512, H=8) on 8 TRN2 NeuronCores.

Sharding: head/tensor parallel x data parallel. Core c (0..7) handles
batch b = c // 4 and head pair g = c % 4 (heads 2g, 2g+1). Each core
computes, for its batch and its two heads: the QKV projections, causal
flash attention over the full sequence, and a partial output projection
against its 128 columns of w_out. The host sums the four partial
[T, D] outputs per batch (the reduce step of the column-parallel
out-projection) and stacks the two batches.

On-chip layout ("transposed flash"): scores are computed as
S^T[k, q] = K^T_tile.T @ Q^T so softmax normalization reduces over the
PSUM partition axis via an appended ones-column on the V stationary
([V | 1]), which yields numerator rows 0..63 and the denominator in
row 64 of the same accumulator. exp() runs on the scalar engine with
the 1/sqrt(HD) scale folded in; causal masking is memset + affine_select
on the diagonal-straddling tiles only. All matmuls use float32r
float16 operands (full PE rate, FWL weight loads, ~5e-4 rel err; accumulation is fp32 in PSUM).
"""

import sys
import types
from contextlib import ExitStack

import numpy as np

B, T, D = 2, 4096, 512
H, HD = 8, 64
QB = 512  # query block (columns of S^T tiles)
KT = 128  # key tile (partition rows of S^T tiles)
NQB = T // QB  # 8
NKT = T // KT  # 32
EC = D // 128  # 4 contraction chunks of 128 over the model dim


def _install_ntff_shim():
    """Make ``antenv.axon_hooks`` importable so run_bass_kernel_spmd's
    trace path never crashes (and actually profiles when the axon .so
    supports it). Degrades to trace-skipped if anything is missing."""
    if "antenv.axon_hooks" in sys.modules:
        return
    mod = types.ModuleType("antenv.axon_hooks")
    mod._hook = None
    mod.set_axon_ntff_profile_hook = lambda h: setattr(mod, "_hook", h)
    mod.get_axon_ntff_profile_hook = lambda: mod._hook
    sys.modules["antenv.axon_hooks"] = mod
    try:
        import antenv

        antenv.axon_hooks = mod
    except ImportError:
        pass
    try:
        from trn_agent_boot.trn_boot import _ntff_profile_via_ctypes

        mod._hook = _ntff_profile_via_ctypes("/opt/axon/libaxon_pjrt.so")
    except Exception:
        pass


_NC_CACHE = {}


def _build():
    import concourse.bass as bass
    import concourse.mybir as mybir
    import concourse.tile as tile
    from concourse import bacc

    F32 = mybir.dt.float32
    F32R = mybir.dt.float16
    EXP = mybir.ActivationFunctionType.Exp
    GE = mybir.AluOpType.is_ge

    nc = bacc.Bacc(None, target_bir_lowering=False)
    xT_in = nc.declare_dram_parameter("xT", [D, T], F32, isOutput=False)
    wqT_in = nc.declare_dram_parameter("wqT", [D, 128], F32, isOutput=False)
    wkT_in = nc.declare_dram_parameter("wkT", [D, 128], F32, isOutput=False)
    wvT_in = nc.declare_dram_parameter("wvT", [D, 128], F32, isOutput=False)
    woT_in = nc.declare_dram_parameter("woT", [128, D], F32, isOutput=False)
    y_out = nc.declare_dram_parameter("y", [T, D], F32, isOutput=True)

    with tile.TileContext(nc) as tc, ExitStack() as ctx:
        const = ctx.enter_context(tc.tile_pool(name="const", bufs=1))
        big = ctx.enter_context(tc.tile_pool(name="big", bufs=1))

        ident = const.tile([128, 128], F32)
        nc.gpsimd.memset(ident[:], 0.0)
        nc.gpsimd.affine_select(
            out=ident[:],
            in_=ident[:],
            compare_op=mybir.AluOpType.not_equal,
            fill=1.0,
            base=0,
            pattern=[[-1, 128]],
            channel_multiplier=1,
        )

        # Warm the scalar engine's exp table during the prologue so the
        # first real exp doesn't stall the attention pipeline ~2.7us.
        warm = const.tile([1, 1], F32)
        nc.scalar.activation(warm[:], ident[0:1, 0:1], EXP, scale=1.0)

        # Causal masks for the 4 diagonal-straddling k-tile offsets.
        # mask[k, d, q] = 1.0 iff (q - k - d*KT) >= 0, else 0.
        cmask = const.tile([128, QB // KT, QB], F32R)
        nc.gpsimd.memset(cmask[:], 1.0)
        for di in range(QB // KT):
            nc.gpsimd.affine_select(
                out=cmask[:, di, :],
                in_=cmask[:, di, :],
                compare_op=GE,
                fill=0.0,
                base=-di * KT,
                pattern=[[1, QB]],
                channel_multiplier=-1,
            )

        # ---- prologue: weights, x^T, projections, V tiles ----
        qT_r = big.tile([128, T], F32R)  # partitions: head A d 0-63, head B 64-127
        # K^T per head, zero-padded to full 128-partition stationaries so QK
        # matmuls contract over all 128 rows (keeps the PE array fully active;
        # 64-deep contractions trip the activity monitor's 50% duty clamp).
        kT_p = big.tile([128, 2, T], F32R)
        xT_r = big.tile([128, EC, T], F32R)
        v_t = big.tile([128, NKT, 2, 65], F32R)
        wo_r = const.tile([128, D], F32R)

        with (
            tc.tile_pool(name="stage", bufs=1) as stage,
            tc.tile_pool(name="xstage", bufs=2) as xstage,
            tc.tile_pool(name="proj_ps", bufs=2, space="PSUM") as proj_ps,
        ):
            nc.vector.memset(kT_p[:], 0.0)
            w_f = stage.tile([128, 3, EC, 128], F32)  # staging for wq/wk/wv T
            nc.sync.dma_start(w_f[:, 0], wqT_in.rearrange("(c p) d -> p c d", p=128))
            nc.sync.dma_start(w_f[:, 1], wkT_in.rearrange("(c p) d -> p c d", p=128))
            nc.sync.dma_start(w_f[:, 2], wvT_in.rearrange("(c p) d -> p c d", p=128))
            w_r = stage.tile([128, 3, EC, 128], F32R)
            nc.vector.tensor_copy(w_r[:], w_f[:])

            wo_f = stage.tile([128, D], F32)
            nc.sync.dma_start(wo_f[:], woT_in[:])
            nc.vector.tensor_copy(wo_r[:], wo_f[:])

            # x^T: per-e-chunk DMAs (16KB contiguous runs) + rounding casts
            for c in range(EC):
                xs = xstage.tile([128, T], F32, tag="xs")
                nc.sync.dma_start(xs[:], xT_in[bass.ts(c, 128), :])
                nc.vector.tensor_copy(xT_r[:, c, :], xs[:])

            # projections: Q^T, K^T (f32r) and V^T (f32)
            vT_f = stage.tile([128, T], F32)
            for wi in range(3):
                for s in range(NQB):
                    pt = proj_ps.tile([128, QB], F32, tag="proj")
                    for c in range(EC):
                        nc.tensor.matmul(
                            pt[:],
                            w_r[:, wi, c],
                            xT_r[:, c, bass.ts(s, QB)],
                            start=(c == 0),
                            stop=(c == EC - 1),
                        )
                    if wi == 0:
                        nc.vector.tensor_copy(qT_r[:, bass.ts(s, QB)], pt[:])
                    elif wi == 2:
                        nc.vector.tensor_copy(vT_f[:, bass.ts(s, QB)], pt[:])
                    else:
                        nc.vector.tensor_copy(
                            kT_p[0:64, 0, bass.ts(s, QB)], pt[0:64, :]
                        )
                        nc.vector.tensor_copy(
                            kT_p[64:128, 1, bass.ts(s, QB)], pt[64:128, :]
                        )

            # ---- V tiles: [128 k, 65] per (head, k-tile), ones col appended
            # (f32r memset is not a valid ISA op; round a f32 ones tile)
            ones_f = stage.tile([128, NKT], F32)
            nc.vector.memset(ones_f[:], 1.0)
            for h in range(2):
                nc.vector.tensor_copy(
                    v_t[:, :, h, 64:65], ones_f[:].unsqueeze(2)
                )
            for t in range(NKT):
                for h in range(2):
                    pv = proj_ps.tile([128, 64], F32, tag="vtp")
                    nc.tensor.transpose(
                        pv[:],
                        vT_f[bass.ts(h, 64), bass.ts(t, KT)],
                        ident[bass.ts(h, 64), bass.ts(h, 64)],
                    )
                    nc.vector.tensor_copy(v_t[:, t, h, 0:64], pv[:])

        # ---- attention + out-projection ----
        s_ps = ctx.enter_context(tc.tile_pool(name="s_ps", bufs=4, space="PSUM"))
        acc_ps = ctx.enter_context(tc.tile_pool(name="acc_ps", bufs=2, space="PSUM"))
        y_ps = ctx.enter_context(tc.tile_pool(name="y_ps", bufs=2, space="PSUM"))
        p_sb = ctx.enter_context(tc.tile_pool(name="p_sb", bufs=6))
        a_sb = ctx.enter_context(tc.tile_pool(name="a_sb", bufs=2))
        d_sb = ctx.enter_context(tc.tile_pool(name="d_sb", bufs=2))
        y_sb = ctx.enter_context(tc.tile_pool(name="y_sb", bufs=3))

        scale = 1.0 / float(np.sqrt(HD))
        DEPTH = 2  # how many k-tiles QK runs ahead of AV

        def emit_outproj(aT_prev, Jp, sub):
            yp = y_ps.tile([128, D], F32)
            nc.tensor.matmul(
                yp[:],
                aT_prev[:, bass.ts(sub, 128)],
                wo_r[:],
                start=True,
                stop=True,
            )
            ysb = y_sb.tile([128, D], F32)
            nc.vector.tensor_copy(ysb[:], yp[:])
            nc.sync.dma_start(
                y_out[bass.ds(Jp * QB + sub * 128, 128), :], ysb[:]
            )

        pending = []  # deferred out-proj thunks for the previous q-block
        for J in range(NQB):
            aT_st = a_sb.tile([128, QB], F32R)
            for h in range(2):
                acc = acc_ps.tile([65, QB], F32)
                ktiles = (J + 1) * (QB // KT)
                pts = [None] * ktiles
                for t in range(ktiles + DEPTH):
                    if t < ktiles:
                        diag = t * KT - J * QB  # >=0 on diagonal tiles
                        lo = max(diag, 0)  # first valid q column
                        st = s_ps.tile([128, QB], F32)
                        nc.tensor.matmul(
                            st[:, lo:QB],
                            kT_p[:, h, bass.ts(t, KT)],
                            qT_r[:, bass.ds(J * QB + lo, QB - lo)],
                            start=True,
                            stop=True,
                        )
                        pt = p_sb.tile([128, QB], F32R)
                        nc.scalar.activation(
                            pt[:, lo:QB], st[:, lo:QB], EXP, scale=scale
                        )
                        if diag >= 0:
                            nc.vector.tensor_mul(
                                pt[:, diag : diag + KT],
                                pt[:, diag : diag + KT],
                                cmask[:, diag // KT, diag : diag + KT],
                            )
                        pts[t] = (pt, lo)
                        if pending and h == 0 and t % 2 == 1:
                            pending.pop(0)()
                    if t >= DEPTH:
                        pt_prev, lo_prev = pts[t - DEPTH]
                        nc.tensor.matmul(
                            acc[:, lo_prev:QB],
                            v_t[:, t - DEPTH, h],
                            pt_prev[:, lo_prev:QB],
                            start=(t == DEPTH),
                            stop=(t == ktiles + DEPTH - 1),
                        )
                # numerator rows 0..63, denominator row 64 -> divide
                drow = d_sb.tile([1, QB], F32, tag="drow")
                nc.vector.tensor_copy(drow[:], acc[64:65, :])
                bc = d_sb.tile([64, QB], F32, tag="bc")
                nc.gpsimd.partition_broadcast(bc[:], drow[:])
                rbc = d_sb.tile([64, QB], F32, tag="rbc")
                nc.vector.reciprocal_approx_fast(out=rbc[:], in_=bc[:])
                nc.vector.tensor_mul(
                    aT_st[bass.ts(h, 64), :], acc[0:64, :], rbc[:]
                )
            for fl in pending:  # anything not yet emitted (early small blocks)
                fl()
            pending = [
                (lambda a=aT_st, Jp=J, sb=sub: emit_outproj(a, Jp, sb))
                for sub in range(QB // KT)
            ]
        for fl in pending:
            fl()

    nc.compile()
    return nc


def get_nc():
    if "nc" not in _NC_CACHE:
        _NC_CACHE["nc"] = _build()
    return _NC_CACHE["nc"]


def make_in_maps(x, w_qkv, w_out):
    x = np.ascontiguousarray(np.asarray(x), dtype=np.float32)
    w_qkv = np.ascontiguousarray(np.asarray(w_qkv), dtype=np.float32)
    w_out = np.ascontiguousarray(np.asarray(w_out), dtype=np.float32)
    in_maps = []
    for c in range(8):
        b, g = divmod(c, 4)
        rows = slice(g * 128, (g + 1) * 128)
        in_maps.append(
            {
                "xT": np.ascontiguousarray(x[b].T),
                "wqT": np.ascontiguousarray(w_qkv[rows, :].T),
                "wkT": np.ascontiguousarray(w_qkv[512 + g * 128 : 512 + (g + 1) * 128, :].T),
                "wvT": np.ascontiguousarray(w_qkv[1024 + g * 128 : 1024 + (g + 1) * 128, :].T),
                "woT": np.ascontiguousarray(w_out[:, rows].T),
            }
        )
    return in_maps


def combine_results(results):
    y = np.zeros((B, T, D), dtype=np.float32)
    for c, r in enumerate(results):
        y[c // 4] += r["y"]
    return y


def kernel(x, w_qkv, w_out, trace=False):
    _install_ntff_shim()
    from concourse.bass_utils import run_bass_kernel_spmd

    nc = get_nc()
    in_maps = make_in_maps(x, w_qkv, w_out)
    r = run_bass_kernel_spmd(nc, in_maps, core_ids=list(range(8)), trace=trace)
    y = combine_results(r.results)
    if trace:
        return y, r
    return y
